# revision 5
# baseline (speedup 1.0000x reference)
"""Trainium2 Bass kernel for ContinualLoraMoeInjectedLinear (moe_routing).

Computation (task_id = tid, static):
    logits = x @ route[tid-1][:, :tid-1]          # [B,S,J], J = tid-1
    omega  = softmax(logits, -1)
    z_j    = x @ down_j                           # rank R=8, j = 0..tid-1
    delta  = sum_{j<J} omega_j * z_j @ up_j + z_{tid-1} @ up_{tid-1}
    out    = x @ W.T + delta

Strategy: data-parallel over the 8192 token rows across 8 cores (1024 each).
Per core: x^T kept resident in SBUF; W streamed once in 1 MB chunks and
transposed on the PE; main matmul computes out^T tiles [128 dout, 512 tok]
accumulating over K in PSUM (fp32r = full PE rate); the tiny LoRA delta is
fused as one extra rank-32 matmul into the same PSUM accumulator. Output is
written transposed ([DOUT, M_core]) and untransposed on the host.
"""

import numpy as np

import concourse.bass as bass  # noqa: F401
import concourse.mybir as mybir
import concourse.tile as tile
from concourse import bacc
from concourse.masks import make_identity

F32 = mybir.dt.float32
F32R = mybir.dt.float32r

N_CORES = 8


def build_bass(
    m_core=1024,  # token rows per core
    K=4096,  # d_in
    DOUT=4096,  # d_out
    J=3,  # number of gated (previous) experts
    E=4,  # number of experts incl. current (= tid)
    R=8,  # lora rank
    NSL=256,  # dout slice width
    TSL=512,  # token slice width (matmul free dim)
    KC=1024,  # k columns per W dma chunk
    mm_f32r=True,
):
    AUXF = J + R * E  # aux columns: route logits + all z
    AUXF2 = AUXF + (AUXF % 2)  # fp32r matmul needs even free size
    AUXP = 64  # host-padded aux width
    RK = R * E  # delta contraction rank
    assert AUXF <= AUXP
    KT = K // 128  # k tiles
    MT = m_core // 128  # token tiles
    NS = DOUT // NSL  # dout slices
    NDC = NSL // 128  # dout chunks per slice
    NTS = m_core // TSL  # token slices
    NKC = K // KC  # w dma chunks per slice
    KPC = KC // 128  # k tiles per w chunk

    nc = bacc.Bacc("TRN2", target_bir_lowering=False, debug=False)

    xs = nc.dram_tensor("xs", [m_core, K], F32, kind="ExternalInput")
    w = nc.dram_tensor("w", [DOUT, K], F32, kind="ExternalInput")
    aux = nc.dram_tensor("aux", [K, AUXP], F32, kind="ExternalInput")
    upall = nc.dram_tensor("upall", [RK, DOUT], F32, kind="ExternalInput")
    outt = nc.dram_tensor("outt", [DOUT, m_core], F32, kind="ExternalOutput")

    MMDT = F32R if mm_f32r else F32

    with tile.TileContext(nc) as tc:
        with (
            tc.tile_pool(name="persist", bufs=1) as persist,
            tc.tile_pool(name="psum_acc", bufs=6, space="PSUM") as accp,
            tc.tile_pool(name="psum_stage", bufs=2, space="PSUM") as stagep,
            tc.tile_pool(name="smalls", bufs=3) as smalls,
        ):
            ident = persist.tile([128, 128], F32, tag="ident")
            make_identity(nc, ident)
            xt_all = persist.tile([128, KT, m_core], MMDT, tag="xt")
            up_sb = persist.tile([RK, DOUT], MMDT, tag="up")
            nc.sync.dma_start(out=up_sb, in_=upall[:, :].bitcast(MMDT))
            zwt_sb = persist.tile([RK, m_core], MMDT, tag="zwt")

            # ---------------- prologue: x load + transpose, routing, Zw^T ----
            with tc.tile_pool(name="prol", bufs=2) as prol:
                aux_sb = prol.tile([128, KT, AUXP], MMDT, tag="aux", bufs=1)
                nc.sync.dma_start(
                    out=aux_sb,
                    in_=aux[:, :].rearrange("(kt p) f -> p kt f", p=128).bitcast(MMDT),
                )
                for m in range(MT):
                    xn = prol.tile([128, K], F32, tag="xn")
                    nc.sync.dma_start(out=xn, in_=xs[m * 128 : (m + 1) * 128, :])
                    for k in range(KT):
                        st = stagep.tile([128, 128], F32, tag="stage")
                        nc.tensor.transpose(st, xn[:, k * 128 : (k + 1) * 128], ident)
                        nc.vector.tensor_copy(
                            xt_all[:, k, m * 128 : (m + 1) * 128], st
                        )
                    # routing + z: XA[m] = x_m @ aux  (accumulate over k)
                    xa_ps = stagep.tile([128, AUXP], F32, tag="stage")
                    for k in range(KT):
                        nc.tensor.matmul(
                            xa_ps[:, :AUXF2],
                            xt_all[:, k, m * 128 : (m + 1) * 128],
                            aux_sb[:, k, :AUXF2],
                            start=(k == 0),
                            stop=(k == KT - 1),
                        )
                    xa_sb = smalls.tile([128, AUXF], F32, tag="xa")
                    nc.vector.tensor_copy(xa_sb, xa_ps[:, :AUXF])
                    # softmax over J logits (no max-sub: |logits| small by scale)
                    ex = smalls.tile([128, J], F32, tag="ex")
                    nc.scalar.activation(ex, xa_sb[:, :J], mybir.ActivationFunctionType.Exp)
                    esum = smalls.tile([128, 1], F32, tag="esum")
                    nc.vector.reduce_sum(out=esum, in_=ex, axis=mybir.AxisListType.X)
                    rinv = smalls.tile([128, 1], F32, tag="rinv")
                    nc.vector.reciprocal(rinv, esum)
                    om = smalls.tile([128, J], F32, tag="om")
                    nc.vector.tensor_scalar_mul(om, ex, rinv)
                    # Zw: gated z for previous experts, raw z for current
                    zw_sb = smalls.tile([128, RK], F32, tag="zw")
                    for j in range(J):
                        nc.vector.tensor_scalar_mul(
                            zw_sb[:, j * R : (j + 1) * R],
                            xa_sb[:, J + j * R : J + (j + 1) * R],
                            om[:, j : j + 1],
                        )
                    for j in range(J, E):
                        nc.vector.tensor_copy(
                            zw_sb[:, j * R : (j + 1) * R],
                            xa_sb[:, J + j * R : J + (j + 1) * R],
                        )
                    # Zw^T [RK, 128] at token column m
                    zt = stagep.tile([RK, 128], F32, tag="stage")
                    nc.tensor.transpose(zt, zw_sb, ident)
                    nc.vector.tensor_copy(
                        zwt_sb[:, m * 128 : (m + 1) * 128], zt
                    )

            # ---------------- main: out^T = W @ x^T + up^T-delta -------------
            with (
                tc.tile_pool(name="wn", bufs=3) as wnp,
                tc.tile_pool(name="wt", bufs=6) as wtp,
                tc.tile_pool(name="osb", bufs=4) as osbp,
            ):
                for n in range(NS):
                    accs = {}
                    for d in range(NDC):
                        for t in range(NTS):
                            accs[(d, t)] = accp.tile(
                                [128, TSL], F32, tag="acc", name=f"acc_{n}_{d}_{t}"
                            )
                    for kc in range(NKC):
                        wn = wnp.tile([128, NDC, KC], F32, tag="wn")
                        nc.sync.dma_start(
                            out=wn,
                            in_=w[
                                n * NSL : (n + 1) * NSL,
                                kc * KC : (kc + 1) * KC,
                            ].rearrange("(db p) k -> p db k", p=128),
                        )
                        for kk in range(KPC):
                            k = kc * KPC + kk
                            wt = wtp.tile([128, NDC, 128], MMDT, tag="wt")
                            st = stagep.tile([128, NDC, 128], F32, tag="stage")
                            for d in range(NDC):
                                nc.tensor.transpose(
                                    st[:, d, :],
                                    wn[:, d, kk * 128 : (kk + 1) * 128],
                                    ident,
                                )
                            nc.vector.tensor_copy(wt, st)
                            for d in range(NDC):
                                for t in range(NTS):
                                    nc.tensor.matmul(
                                        accs[(d, t)],
                                        wt[:, d, :],
                                        xt_all[:, k, t * TSL : (t + 1) * TSL],
                                        start=(k == 0),
                                        stop=False,
                                    )
                    for d in range(NDC):
                        for t in range(NTS):
                            # fused lora delta: rank-RK update into the same bank
                            nc.tensor.matmul(
                                accs[(d, t)],
                                up_sb[:, n * NSL + d * 128 : n * NSL + (d + 1) * 128],
                                zwt_sb[:, t * TSL : (t + 1) * TSL],
                                start=False,
                                stop=True,
                            )
                            osb = osbp.tile([128, TSL], F32, tag="osb")
                            nc.vector.tensor_copy(osb, accs[(d, t)])
                            nc.sync.dma_start(
                                out=outt[
                                    n * NSL + d * 128 : n * NSL + (d + 1) * 128,
                                    t * TSL : (t + 1) * TSL,
                                ],
                                in_=osb,
                            )
    nc.compile()
    return nc


_CACHE: dict = {}


def _get_nc(key, **kw):
    if key not in _CACHE:
        _CACHE[key] = build_bass(**kw)
    return _CACHE[key]


def _host_prep(x, W, lora_down, lora_up, lora_route, tid):
    """Build per-core input maps. Tiny host work: flatten/shard x, concat lora params."""
    B, S, K = x.shape
    DOUT = W.shape[0]
    J, E, R = tid - 1, tid, lora_down.shape[2]
    xf = np.ascontiguousarray(x.reshape(B * S, K), dtype=np.float32)
    m_core = (B * S) // N_CORES
    # aux = [route_{tid-1}[:, :J] | down_0 | ... | down_{tid-1}]  -> [K, J+R*E] pad 64
    aux = np.zeros((K, 64), dtype=np.float32)
    aux[:, :J] = lora_route[tid - 1][:, :J]
    for j in range(E):
        aux[:, J + j * R : J + (j + 1) * R] = lora_down[j]
    upall = np.ascontiguousarray(
        lora_up[:E].reshape(E * R, DOUT), dtype=np.float32
    )
    w = np.ascontiguousarray(W, dtype=np.float32)
    in_maps = [
        {
            "xs": xf[c * m_core : (c + 1) * m_core],
            "w": w,
            "aux": aux,
            "upall": upall,
        }
        for c in range(N_CORES)
    ]
    return in_maps, (B, S, K, DOUT, m_core)


def kernel(x, W, lora_down, lora_up, lora_route, task_id):
    from concourse.bass_utils import run_bass_kernel_spmd

    tid = int(task_id)
    in_maps, (B, S, K, DOUT, m_core) = _host_prep(
        np.asarray(x), np.asarray(W), np.asarray(lora_down),
        np.asarray(lora_up), np.asarray(lora_route), tid,
    )
    nc = _get_nc(
        ("main", m_core, K, DOUT, tid),
        m_core=m_core, K=K, DOUT=DOUT, J=tid - 1, E=tid,
        R=lora_down.shape[2],
    )
    res = run_bass_kernel_spmd(nc, in_maps, core_ids=list(range(N_CORES)))
    out = np.empty((B * S, DOUT), dtype=np.float32)
    for c in range(N_CORES):
        out[c * m_core : (c + 1) * m_core] = res.results[c]["outt"].T
    return out.reshape(B, S, DOUT)


# revision 6
# speedup vs baseline: 200.8254x; 200.8254x over previous
"""Trainium2 Bass kernel for ContinualLoraMoeInjectedLinear (moe_routing).

Computation (task_id = tid, static):
    logits = x @ route[tid-1][:, :tid-1]          # [B,S,J], J = tid-1
    omega  = softmax(logits, -1)
    z_j    = x @ down_j                           # rank R=8, j = 0..tid-1
    delta  = sum_{j<J} omega_j * z_j @ up_j + z_{tid-1} @ up_{tid-1}
    out    = x @ W.T + delta

Strategy: data-parallel over the 8192 token rows across 8 cores (1024 each).
Per core: x^T kept resident in SBUF; W streamed once in 1 MB chunks and
transposed on the PE; main matmul computes out^T tiles [128 dout, 512 tok]
accumulating over K in PSUM (fp32r = full PE rate); the tiny LoRA delta is
fused as one extra rank-32 matmul into the same PSUM accumulator. Output is
written transposed ([DOUT, M_core]) and untransposed on the host.
"""

import numpy as np

import concourse.bass as bass  # noqa: F401
import concourse.mybir as mybir
import concourse.tile as tile
from concourse import bacc
from concourse.masks import make_identity

F32 = mybir.dt.float32
F32R = mybir.dt.float32r

N_CORES = 8


def build_bass(
    m_core=1024,  # token rows per core
    K=4096,  # d_in
    DOUT=4096,  # d_out
    J=3,  # number of gated (previous) experts
    E=4,  # number of experts incl. current (= tid)
    R=8,  # lora rank
    NSL=256,  # dout slice width
    TSL=512,  # token slice width (matmul free dim)
    KC=1024,  # k columns per W dma chunk
    mm_f32r=True,
    reps=1,  # on-device repeat count (differential HW timing)
):
    AUXF = J + R * E  # aux columns: route logits + all z
    AUXF2 = AUXF + (AUXF % 2)  # fp32r matmul needs even free size
    AUXP = 64  # host-padded aux width
    RK = R * E  # delta contraction rank
    assert AUXF <= AUXP
    KT = K // 128  # k tiles
    MT = m_core // 128  # token tiles
    NS = DOUT // NSL  # dout slices
    NDC = NSL // 128  # dout chunks per slice
    NTS = m_core // TSL  # token slices
    NKC = K // KC  # w dma chunks per slice
    KPC = KC // 128  # k tiles per w chunk

    nc = bacc.Bacc("TRN2", target_bir_lowering=False, debug=False)

    xs = nc.dram_tensor("xs", [m_core, K], F32, kind="ExternalInput")
    w = nc.dram_tensor("w", [DOUT, K], F32, kind="ExternalInput")
    aux = nc.dram_tensor("aux", [K, AUXP], F32, kind="ExternalInput")
    upall = nc.dram_tensor("upall", [RK, DOUT], F32, kind="ExternalInput")
    outt = nc.dram_tensor("outt", [DOUT, m_core], F32, kind="ExternalOutput")

    MMDT = F32R if mm_f32r else F32

    import contextlib

    with tile.TileContext(nc) as tc:
        with (
            tc.For_i(0, reps, 1) if reps > 1 else contextlib.nullcontext(),
            tc.tile_pool(name="persist", bufs=1) as persist,
            tc.tile_pool(name="psum_acc", bufs=6, space="PSUM") as accp,
            tc.tile_pool(name="psum_stage", bufs=2, space="PSUM") as stagep,
            tc.tile_pool(name="smalls", bufs=3) as smalls,
        ):
            ident = persist.tile([128, 128], F32, tag="ident")
            make_identity(nc, ident)
            xt_all = persist.tile([128, KT, m_core], MMDT, tag="xt")
            up_sb = persist.tile([RK, DOUT], MMDT, tag="up")
            nc.sync.dma_start(out=up_sb, in_=upall[:, :].bitcast(MMDT))
            zwt_sb = persist.tile([RK, m_core], MMDT, tag="zwt")

            # ---------------- prologue: x load + transpose, routing, Zw^T ----
            with tc.tile_pool(name="prol", bufs=2) as prol:
                aux_sb = prol.tile([128, KT, AUXP], MMDT, tag="aux", bufs=1)
                nc.sync.dma_start(
                    out=aux_sb,
                    in_=aux[:, :].rearrange("(kt p) f -> p kt f", p=128).bitcast(MMDT),
                )
                for m in range(MT):
                    xn = prol.tile([128, K], F32, tag="xn")
                    nc.sync.dma_start(out=xn, in_=xs[m * 128 : (m + 1) * 128, :])
                    for k in range(KT):
                        st = stagep.tile([128, 128], F32, tag="stage")
                        nc.tensor.transpose(st, xn[:, k * 128 : (k + 1) * 128], ident)
                        nc.vector.tensor_copy(
                            xt_all[:, k, m * 128 : (m + 1) * 128], st
                        )
                    # routing + z: XA[m] = x_m @ aux  (accumulate over k)
                    xa_ps = stagep.tile([128, AUXP], F32, tag="stage")
                    for k in range(KT):
                        nc.tensor.matmul(
                            xa_ps[:, :AUXF2],
                            xt_all[:, k, m * 128 : (m + 1) * 128],
                            aux_sb[:, k, :AUXF2],
                            start=(k == 0),
                            stop=(k == KT - 1),
                        )
                    xa_sb = smalls.tile([128, AUXF], F32, tag="xa")
                    nc.vector.tensor_copy(xa_sb, xa_ps[:, :AUXF])
                    # softmax over J logits (no max-sub: |logits| small by scale)
                    ex = smalls.tile([128, J], F32, tag="ex")
                    nc.scalar.activation(ex, xa_sb[:, :J], mybir.ActivationFunctionType.Exp)
                    esum = smalls.tile([128, 1], F32, tag="esum")
                    nc.vector.reduce_sum(out=esum, in_=ex, axis=mybir.AxisListType.X)
                    rinv = smalls.tile([128, 1], F32, tag="rinv")
                    nc.vector.reciprocal(rinv, esum)
                    om = smalls.tile([128, J], F32, tag="om")
                    nc.vector.tensor_scalar_mul(om, ex, rinv)
                    # Zw: gated z for previous experts, raw z for current
                    zw_sb = smalls.tile([128, RK], F32, tag="zw")
                    for j in range(J):
                        nc.vector.tensor_scalar_mul(
                            zw_sb[:, j * R : (j + 1) * R],
                            xa_sb[:, J + j * R : J + (j + 1) * R],
                            om[:, j : j + 1],
                        )
                    for j in range(J, E):
                        nc.vector.tensor_copy(
                            zw_sb[:, j * R : (j + 1) * R],
                            xa_sb[:, J + j * R : J + (j + 1) * R],
                        )
                    # Zw^T [RK, 128] at token column m
                    zt = stagep.tile([RK, 128], F32, tag="stage")
                    nc.tensor.transpose(zt, zw_sb, ident)
                    nc.vector.tensor_copy(
                        zwt_sb[:, m * 128 : (m + 1) * 128], zt
                    )

            # ---------------- main: out^T = W @ x^T + up^T-delta -------------
            with (
                tc.tile_pool(name="wn", bufs=3) as wnp,
                tc.tile_pool(name="wt", bufs=6) as wtp,
                tc.tile_pool(name="osb", bufs=4) as osbp,
            ):
                for n in range(NS):
                    accs = {}
                    for d in range(NDC):
                        for t in range(NTS):
                            accs[(d, t)] = accp.tile(
                                [128, TSL], F32, tag="acc", name=f"acc_{n}_{d}_{t}"
                            )
                    for kc in range(NKC):
                        wn = wnp.tile([128, NDC, KC], F32, tag="wn")
                        nc.sync.dma_start(
                            out=wn,
                            in_=w[
                                n * NSL : (n + 1) * NSL,
                                kc * KC : (kc + 1) * KC,
                            ].rearrange("(db p) k -> p db k", p=128),
                        )
                        for kk in range(KPC):
                            k = kc * KPC + kk
                            wt = wtp.tile([128, NDC, 128], MMDT, tag="wt")
                            st = stagep.tile([128, NDC, 128], F32, tag="stage")
                            for d in range(NDC):
                                nc.tensor.transpose(
                                    st[:, d, :],
                                    wn[:, d, kk * 128 : (kk + 1) * 128],
                                    ident,
                                )
                            nc.vector.tensor_copy(wt, st)
                            for d in range(NDC):
                                for t in range(NTS):
                                    nc.tensor.matmul(
                                        accs[(d, t)],
                                        wt[:, d, :],
                                        xt_all[:, k, t * TSL : (t + 1) * TSL],
                                        start=(k == 0),
                                        stop=False,
                                    )
                    for d in range(NDC):
                        for t in range(NTS):
                            # fused lora delta: rank-RK update into the same bank
                            nc.tensor.matmul(
                                accs[(d, t)],
                                up_sb[:, n * NSL + d * 128 : n * NSL + (d + 1) * 128],
                                zwt_sb[:, t * TSL : (t + 1) * TSL],
                                start=False,
                                stop=True,
                            )
                            osb = osbp.tile([128, TSL], F32, tag="osb")
                            nc.vector.tensor_copy(osb, accs[(d, t)])
                            nc.sync.dma_start(
                                out=outt[
                                    n * NSL + d * 128 : n * NSL + (d + 1) * 128,
                                    t * TSL : (t + 1) * TSL,
                                ],
                                in_=osb,
                            )
    nc.compile()
    return nc


_CACHE: dict = {}


def _get_nc(key, **kw):
    if key not in _CACHE:
        _CACHE[key] = build_bass(**kw)
    return _CACHE[key]


def _host_prep(x, W, lora_down, lora_up, lora_route, tid):
    """Build per-core input maps. Tiny host work: flatten/shard x, concat lora params."""
    B, S, K = x.shape
    DOUT = W.shape[0]
    J, E, R = tid - 1, tid, lora_down.shape[2]
    xf = np.ascontiguousarray(x.reshape(B * S, K), dtype=np.float32)
    m_core = (B * S) // N_CORES
    # aux = [route_{tid-1}[:, :J] | down_0 | ... | down_{tid-1}]  -> [K, J+R*E] pad 64
    aux = np.zeros((K, 64), dtype=np.float32)
    aux[:, :J] = lora_route[tid - 1][:, :J]
    for j in range(E):
        aux[:, J + j * R : J + (j + 1) * R] = lora_down[j]
    upall = np.ascontiguousarray(
        lora_up[:E].reshape(E * R, DOUT), dtype=np.float32
    )
    w = np.ascontiguousarray(W, dtype=np.float32)
    in_maps = [
        {
            "xs": xf[c * m_core : (c + 1) * m_core],
            "w": w,
            "aux": aux,
            "upall": upall,
        }
        for c in range(N_CORES)
    ]
    return in_maps, (B, S, K, DOUT, m_core)


def kernel(x, W, lora_down, lora_up, lora_route, task_id):
    from concourse.bass_utils import run_bass_kernel_spmd

    tid = int(task_id)
    in_maps, (B, S, K, DOUT, m_core) = _host_prep(
        np.asarray(x), np.asarray(W), np.asarray(lora_down),
        np.asarray(lora_up), np.asarray(lora_route), tid,
    )
    nc = _get_nc(
        ("main", m_core, K, DOUT, tid),
        m_core=m_core, K=K, DOUT=DOUT, J=tid - 1, E=tid,
        R=lora_down.shape[2],
    )
    res = run_bass_kernel_spmd(nc, in_maps, core_ids=list(range(N_CORES)))
    out = np.empty((B * S, DOUT), dtype=np.float32)
    for c in range(N_CORES):
        out[c * m_core : (c + 1) * m_core] = res.results[c]["outt"].T
    return out.reshape(B, S, DOUT)
